# revision 1
# baseline (speedup 1.0000x reference)
"""Trainium2 Bass kernel for CrossAttention (b=4, p=8, n=512, dim=512, 8 heads x 64).

Sharding: the 32 independent (b, p) slices are split 4-per-core across 8
NeuronCores (pure data parallel, no collectives). Weights are replicated.

Host-side prep (inside kernel()): inputs are transposed per-slice to
[dim, n] and cast to bf16, so the device never transposes activations;
weights are cast to bf16 on the host too.

Per-slice device dataflow (all SBUF tiles are [partition, free]):
  - qT = Wq-blocks^T @ xqT, kT likewise; v = xkvT-blocks^T @ Wv  (PE)
  - per head: ST[j, i] = kT_h-block^T @ qT_h -> exp (ACT, scale=1/8) -> PT bf16
    (no max subtraction: scores are ~N(0,1), exp cannot overflow)
  - PV for a head pair is issued column-group-paired so PE overlaps:
    even head outT -> PSUM[0:64] while odd head's l-matmul (ones[128,33])
    lands in PSUM[64:97] of a second bank, and vice versa. l rows at the
    two quadrant bases let a DVE stream_shuffle broadcast l across all 64
    partitions of the head's parity range, all partition-aligned.
  - normalize: outT_h *= 1/l_h (DVE reciprocal + stream_shuffle + mul)
  - final: fin[i, f] = outT-blocks^T @ Wo (+ ones1 x bo) -> fp32 -> DRAM.
"""

from contextlib import ExitStack

import ml_dtypes
import numpy as np

import concourse.bass as bass
import concourse.tile as tile
from concourse import bacc, mybir
from concourse.bass_utils import run_bass_kernel_spmd

F32 = mybir.dt.float32
BF16 = mybir.dt.bfloat16

HEADS = 8
DH = 64
N = 512
DIM = 512
SCALE = DH**-0.5
S = 4  # (b, p) slices per core
N_CORES = 8

SHUF0 = [0] * 32  # stream_shuffle mask: broadcast quadrant partition 0


def _build_body(ctx: ExitStack, tc: tile.TileContext, qT, kvT, wq, wk, wv, wo, bo, out):
    nc = tc.nc

    const = ctx.enter_context(tc.tile_pool(name="const", bufs=1))
    xT = ctx.enter_context(tc.tile_pool(name="xT", bufs=3))
    proj = ctx.enter_context(tc.tile_pool(name="proj", bufs=2))
    ptp = ctx.enter_context(tc.tile_pool(name="ptp", bufs=4))
    outTp = ctx.enter_context(tc.tile_pool(name="outTp", bufs=2))
    rbp = ctx.enter_context(tc.tile_pool(name="rbp", bufs=4))
    finp = ctx.enter_context(tc.tile_pool(name="finp", bufs=2))
    mm_ps = ctx.enter_context(tc.tile_pool(name="mm_ps", bufs=2, space="PSUM"))
    st_ps = ctx.enter_context(tc.tile_pool(name="st_ps", bufs=3, space="PSUM"))
    pv_ps = ctx.enter_context(tc.tile_pool(name="pv_ps", bufs=2, space="PSUM"))
    l_ps = ctx.enter_context(tc.tile_pool(name="l_ps", bufs=1, space="PSUM"))

    # --- weights (already bf16 in DRAM): [512, 512] -> [128, 4*512] ---
    w_sb = {}
    for name, dram in (("wq", wq), ("wk", wk), ("wv", wv), ("wo", wo)):
        w16 = const.tile([128, 4 * 512], BF16, name=f"{name}16")
        nc.sync.dma_start(w16[:], dram.rearrange("(t p) e -> p t e", p=128))
        w_sb[name] = w16
    bo16 = const.tile([1, 512], BF16, name="bo16")
    nc.sync.dma_start(bo16[:], bo.rearrange("(o f) -> o f", o=1))
    ones64 = const.tile([128, 64], BF16, name="ones64")
    nc.gpsimd.memset(ones64[:], 1.0)
    ones1 = const.tile([1, 128], BF16, name="ones1")
    nc.gpsimd.memset(ones1[:], 1.0)
    wq16, wk16, wv16, wo16 = (w_sb[k] for k in ("wq", "wk", "wv", "wo"))

    for s in range(S):
        # --- load pre-transposed bf16 inputs ---
        xqT = xT.tile([128, 4 * 512], BF16, name="xqT")
        nc.sync.dma_start(xqT[:], qT[s].rearrange("(t p) n -> p t n", p=128))
        xkvT = xT.tile([128, 4 * 512], BF16, name="xkvT")
        nc.sync.dma_start(xkvT[:], kvT[s].rearrange("(t p) n -> p t n", p=128))

        # --- projections ---
        qT16 = proj.tile([128, 4 * 512], BF16, name="qT16")
        kT16 = proj.tile([128, 4 * 512], BF16, name="kT16")
        v16 = proj.tile([128, 4 * 512], BF16, name="v16")
        for w16, xt, dst in ((wq16, xqT, qT16), (wk16, xkvT, kT16)):
            for t in range(4):  # output row-block (e)
                ps = mm_ps.tile([128, 512], F32, name="mm_ps")
                for d in range(4):  # contraction block
                    nc.tensor.matmul(
                        ps[:],
                        w16[:, d * 512 + t * 128 : d * 512 + (t + 1) * 128],
                        xt[:, d * 512 : (d + 1) * 512],
                        start=(d == 0),
                        stop=(d == 3),
                    )
                nc.vector.tensor_copy(dst[:, t * 512 : (t + 1) * 512], ps[:])
        for jb in range(4):  # v, normal layout: rows j, free e
            ps = mm_ps.tile([128, 512], F32, name="mm_ps")
            for d in range(4):
                nc.tensor.matmul(
                    ps[:],
                    xkvT[:, d * 512 + jb * 128 : d * 512 + (jb + 1) * 128],
                    wv16[:, d * 512 : (d + 1) * 512],
                    start=(d == 0),
                    stop=(d == 3),
                )
            nc.vector.tensor_copy(v16[:, jb * 512 : (jb + 1) * 512], ps[:])

        # --- attention, head pairs ---
        outT16 = outTp.tile([128, 4 * 512], BF16, name="outT16")
        for tp in range(4):
            h0, h1 = 2 * tp, 2 * tp + 1
            pts = []
            for h, half in ((h0, 0), (h1, 64)):
                kT_h = kT16[half : half + 64, tp * 512 : (tp + 1) * 512]
                qT_h = qT16[half : half + 64, tp * 512 : (tp + 1) * 512]
                pt16 = ptp.tile([128, 4 * 512], BF16, name="pt16")
                for jb in range(4):
                    stt = st_ps.tile([128, 512], F32, name="st_ps")
                    nc.tensor.matmul(
                        stt[:],
                        kT_h[:, jb * 128 : (jb + 1) * 128],
                        qT_h,
                        start=True,
                        stop=True,
                    )
                    nc.scalar.activation(
                        pt16[:, jb * 512 : (jb + 1) * 512],
                        stt[:],
                        mybir.ActivationFunctionType.Exp,
                        scale=SCALE,
                    )
                pts.append(pt16)
            pt_e, pt_o = pts

            # PV: column-group-paired issues so PE overlaps outT with l.
            pv = pv_ps.tile([128, 512], F32, name="pv_ps")
            lps = l_ps.tile([128, 512], F32, name="l_ps")
            for jb in range(4):
                pe_s = pt_e[:, jb * 512 : (jb + 1) * 512]
                po_s = pt_o[:, jb * 512 : (jb + 1) * 512]
                st, sp = (jb == 0), (jb == 3)
                # issue A: even outT (cols 0-63) || odd l-bcast (cols 64-127)
                nc.tensor.matmul(
                    pv[0:64, :],
                    v16[:, jb * 512 + h0 * 64 : jb * 512 + (h0 + 1) * 64],
                    pe_s, start=st, stop=sp, skip_group_check=True,
                )
                nc.tensor.matmul(
                    lps[64:128, :], ones64[:], po_s, start=st, stop=sp,
                    skip_group_check=True,
                )
                # issue B: odd outT (cols 64-127) || even l-bcast (cols 0-63)
                nc.tensor.matmul(
                    pv[64:128, :],
                    v16[:, jb * 512 + h1 * 64 : jb * 512 + (h1 + 1) * 64],
                    po_s, start=st, stop=sp, skip_group_check=True,
                )
                nc.tensor.matmul(
                    lps[0:64, :], ones64[:], pe_s, start=st, stop=sp,
                    skip_group_check=True,
                )
            for h, half in ((h0, 0), (h1, 64)):
                rb1 = rbp.tile([128, 512], F32, name="rb1")
                nc.vector.reciprocal(
                    rb1[half : half + 64, :], lps[half : half + 64, :]
                )
                nc.vector.tensor_mul(
                    outT16[half : half + 64, tp * 512 : (tp + 1) * 512],
                    pv[half : half + 64, :],
                    rb1[half : half + 64, :],
                )

        # --- final projection + bias ---
        fin = finp.tile([128, 4 * 512], F32, name="fin")
        for ib in range(4):
            ps = mm_ps.tile([128, 512], F32, name="mm_ps")
            for t in range(4):
                nc.tensor.matmul(
                    ps[:],
                    outT16[:, t * 512 + ib * 128 : t * 512 + (ib + 1) * 128],
                    wo16[:, t * 512 : (t + 1) * 512],
                    start=(t == 0),
                    stop=False,
                )
            nc.tensor.matmul(ps[:], ones1[:], bo16[:], start=False, stop=True)
            nc.vector.tensor_copy(fin[:, ib * 512 : (ib + 1) * 512], ps[:])
        nc.sync.dma_start(out[s].rearrange("(a p) f -> p a f", p=128), fin[:])


def build_nc():
    nc = bacc.Bacc("TRN2", target_bir_lowering=False, debug=False)
    qT = nc.dram_tensor("qT", [S, DIM, N], BF16, kind="ExternalInput").ap()
    kvT = nc.dram_tensor("kvT", [S, DIM, N], BF16, kind="ExternalInput").ap()
    wq = nc.dram_tensor("wq", [DIM, DIM], BF16, kind="ExternalInput").ap()
    wk = nc.dram_tensor("wk", [DIM, DIM], BF16, kind="ExternalInput").ap()
    wv = nc.dram_tensor("wv", [DIM, DIM], BF16, kind="ExternalInput").ap()
    wo = nc.dram_tensor("wo", [DIM, DIM], BF16, kind="ExternalInput").ap()
    bo = nc.dram_tensor("bo", [DIM], BF16, kind="ExternalInput").ap()
    out = nc.dram_tensor("out", [S, N, DIM], F32, kind="ExternalOutput").ap()
    with tile.TileContext(nc) as tc:
        with ExitStack() as ctx:
            _build_body(ctx, tc, qT, kvT, wq, wk, wv, wo, bo, out)
    nc.compile()
    return nc


_NC = None
BF = ml_dtypes.bfloat16


def make_in_maps(q_in, kv_in, Wq, Wk, Wv, Wo, bo):
    # host-side layout prep: per-slice transpose to [dim, n] + bf16 cast
    q = np.asarray(q_in, dtype=np.float32).reshape(32, N, DIM)
    kv = np.asarray(kv_in, dtype=np.float32).reshape(32, N, DIM)
    qT = np.ascontiguousarray(q.transpose(0, 2, 1)).astype(BF)
    kvT = np.ascontiguousarray(kv.transpose(0, 2, 1)).astype(BF)
    w = {
        "wq": np.asarray(Wq, dtype=np.float32).astype(BF),
        "wk": np.asarray(Wk, dtype=np.float32).astype(BF),
        "wv": np.asarray(Wv, dtype=np.float32).astype(BF),
        "wo": np.asarray(Wo, dtype=np.float32).astype(BF),
        "bo": np.asarray(bo, dtype=np.float32).astype(BF),
    }
    return [
        {"qT": qT[S * c : S * (c + 1)], "kvT": kvT[S * c : S * (c + 1)], **w}
        for c in range(N_CORES)
    ]


def kernel(q_in, kv_in, Wq, Wk, Wv, Wo, bo):
    global _NC
    if _NC is None:
        _NC = build_nc()
    in_maps = make_in_maps(q_in, kv_in, Wq, Wk, Wv, Wo, bo)
    res = run_bass_kernel_spmd(_NC, in_maps, list(range(N_CORES))).results
    out = np.concatenate([res[c]["out"] for c in range(N_CORES)], axis=0)
    return out.reshape(4, 8, N, DIM)



# revision 2
# speedup vs baseline: 1.8540x; 1.8540x over previous
"""Trainium2 Bass kernel for CrossAttention (b=4, p=8, n=512, dim=512, 8 heads x 64).

Sharding: 32 independent (b, p) slices, 4 per core across 8 NeuronCores (pure
data parallel, no collectives). Weights replicated.

Host-side prep: activations transposed per-slice to [dim, n], cast bf16, and
packed with kv into one tensor; the 4 weight matrices packed into one tensor;
bo pre-broadcast to [128, 512] f32.

Device dataflow per slice (PE tile-array concurrency is the point):
  - projections qT/kT ([e, i]) and v ([j, e]) in 128x128 mode; PSUM drained by
    DVE copies.
  - scores per head-pair (2*tp, 2*tp+1): K=64 row-tiled matmuls issued
    alternately at tile_position (0,0)/(64,0) so both heads stream
    concurrently in the 64x128 array config; exp on ACT (scale=1/8, no max
    subtraction: scores ~N(0,1)).
  - PV+l in 128x64 col-tiled mode: per jb, issue pv_even (cols 0-63),
    pv_odd (cols 64-127), l_even, l_odd alternating col groups; pv pair packs
    one PSUM bank (disjoint partitions), l pair another. l rows land aligned
    with their head's pv rows, so normalize is ONE DVE reciprocal + ONE
    tensor_mul over all 128 partitions per pair.
  - final projection 128x128 with bias added during the PSUM drain
    (DVE tensor_add with a pre-broadcast [128,512] bias tile) -> f32 -> DRAM.
  - emission is software-pipelined: slice s+1's projection chunks are
    interleaved into slice s's attention so the PE never waits on ACT.
"""

from contextlib import ExitStack

import ml_dtypes
import numpy as np

import concourse.bass as bass
import concourse.tile as tile
from concourse import bacc, mybir
from concourse.bass_utils import run_bass_kernel_spmd

F32 = mybir.dt.float32
BF16 = mybir.dt.bfloat16

HEADS = 8
DH = 64
N = 512
DIM = 512
SCALE = DH**-0.5
S = 4  # (b, p) slices per core
N_CORES = 8


def _build_body(ctx: ExitStack, tc: tile.TileContext, x, w, bo_b, out):
    nc = tc.nc

    const = ctx.enter_context(tc.tile_pool(name="const", bufs=1))
    xT = ctx.enter_context(tc.tile_pool(name="xT", bufs=2))
    proj = ctx.enter_context(tc.tile_pool(name="proj", bufs=2))
    ptp = ctx.enter_context(tc.tile_pool(name="ptp", bufs=4))
    outTp = ctx.enter_context(tc.tile_pool(name="outTp", bufs=2))
    rbp = ctx.enter_context(tc.tile_pool(name="rbp", bufs=2))
    finp = ctx.enter_context(tc.tile_pool(name="finp", bufs=2))
    mm_ps = ctx.enter_context(tc.tile_pool(name="mm_ps", bufs=2, space="PSUM"))
    st_ps = ctx.enter_context(tc.tile_pool(name="st_ps", bufs=3, space="PSUM"))
    pv_ps = ctx.enter_context(tc.tile_pool(name="pv_ps", bufs=2, space="PSUM"))
    l_ps = ctx.enter_context(tc.tile_pool(name="l_ps", bufs=1, space="PSUM"))

    # --- constants ---
    w16 = const.tile([128, 4 * 2048], BF16, name="w16")
    bo_sb = const.tile([128, 512], F32, name="bo_sb")
    ones64 = const.tile([128, 64], BF16, name="ones64")
    nc.gpsimd.memset(ones64[:], 1.0)

    def wblk(widx, d, lo, sz):
        # rows d-chunk (128 partitions), e-cols [lo, lo+sz) of weight widx
        base = widx * 2048 + d * 512 + lo
        return w16[:, base : base + sz]

    # per-slice persistent tiles
    xt_t = [None] * S

    def emit_x_load(s):
        t = xT.tile([128, 2 * 2048], BF16, name="xt")
        nc.sync.dma_start(t[:, 0:2048], x[s, 0].rearrange("(t p) n -> p t n", p=128))
        nc.sync.dma_start(
            t[:, 2048:4096], x[s, 1].rearrange("(t p) n -> p t n", p=128)
        )
        xt_t[s] = t

    qkv_t = [None] * S

    def proj_chunks(s):
        """12 emission chunks: q.t0-3, k.t0-3 ([e,i] layout), v.jb0-3 ([j,e])."""
        qT16 = proj.tile([128, 4 * 512], BF16, name="qT16")
        kT16 = proj.tile([128, 4 * 512], BF16, name="kT16")
        v16 = proj.tile([128, 4 * 512], BF16, name="v16")
        qkv_t[s] = (qT16, kT16, v16)
        xq = lambda d: xt_t[s][:, d * 512 : (d + 1) * 512]
        xkv = lambda d: xt_t[s][:, 2048 + d * 512 : 2048 + (d + 1) * 512]
        chunks = []

        def mk_qk(widx, t, dst):
            def go():
                ps = mm_ps.tile([128, 512], F32, name="mm_ps")
                for d in range(4):
                    nc.tensor.matmul(
                        ps[:],
                        wblk(widx, d, t * 128, 128),
                        xq(d) if widx == 0 else xkv(d),
                        start=(d == 0),
                        stop=(d == 3),
                    )
                nc.vector.tensor_copy(dst[:, t * 512 : (t + 1) * 512], ps[:])

            return go

        def mk_v(jb):
            def go():
                ps = mm_ps.tile([128, 512], F32, name="mm_ps")
                for d in range(4):
                    nc.tensor.matmul(
                        ps[:],
                        xkv(d)[:, jb * 128 : (jb + 1) * 128],
                        wblk(2, d, 0, 512),
                        start=(d == 0),
                        stop=(d == 3),
                    )
                nc.vector.tensor_copy(v16[:, jb * 512 : (jb + 1) * 512], ps[:])

            return go

        for t in range(4):
            chunks.append(mk_qk(0, t, qT16))
        for t in range(4):
            chunks.append(mk_qk(1, t, kT16))
        for jb in range(4):
            chunks.append(mk_v(jb))
        return chunks

    def attn_chunks(s):
        """S.tp0, S.tp1, PV.tp0, S.tp2, PV.tp1, S.tp3, PV.tp2, PV.tp3,
        F.ib0-3 + store -> 12 chunks, emitted so each PV trails its exps."""
        qT16, kT16, v16 = qkv_t[s]
        outT16 = outTp.tile([128, 4 * 512], BF16, name="outT16")
        fin = finp.tile([128, 4 * 512], F32, name="fin")
        pts = [None] * 4

        def mk_scores(tp):
            def go():
                pt_e = ptp.tile([128, 4 * 512], BF16, name="pt_e")
                pt_o = ptp.tile([128, 4 * 512], BF16, name="pt_o")
                pts[tp] = (pt_e, pt_o)
                cols = slice(tp * 512, (tp + 1) * 512)
                for jb in range(4):
                    for half, pt in ((0, pt_e), (64, pt_o)):
                        kT_h = kT16[half : half + 64, cols]
                        qT_h = qT16[half : half + 64, cols]
                        stt = st_ps.tile([128, 512], F32, name="st_ps")
                        nc.tensor.matmul(
                            stt[:],
                            kT_h[:, jb * 128 : (jb + 1) * 128],
                            qT_h,
                            start=True,
                            stop=True,
                        )
                        nc.scalar.activation(
                            pt[:, jb * 512 : (jb + 1) * 512],
                            stt[:],
                            mybir.ActivationFunctionType.Exp,
                            scale=SCALE,
                        )

            return go

        def mk_pv(tp):
            h0, h1 = 2 * tp, 2 * tp + 1

            def go():
                pt_e, pt_o = pts[tp]
                pvb = pv_ps.tile([128, 512], F32, name="pv_ps")
                lb = l_ps.tile([128, 512], F32, name="l_ps")
                for jb in range(4):
                    st, sp = (jb == 0), (jb == 3)
                    pe_s = pt_e[:, jb * 512 : (jb + 1) * 512]
                    po_s = pt_o[:, jb * 512 : (jb + 1) * 512]
                    nc.tensor.matmul(
                        pvb[0:64, :],
                        v16[:, jb * 512 + h0 * 64 : jb * 512 + (h0 + 1) * 64],
                        pe_s, start=st, stop=sp, skip_group_check=True,
                    )
                    nc.tensor.matmul(
                        pvb[64:128, :],
                        v16[:, jb * 512 + h1 * 64 : jb * 512 + (h1 + 1) * 64],
                        po_s, start=st, stop=sp, skip_group_check=True,
                    )
                    nc.tensor.matmul(
                        lb[0:64, :], ones64[:], pe_s, start=st, stop=sp,
                        skip_group_check=True,
                    )
                    nc.tensor.matmul(
                        lb[64:128, :], ones64[:], po_s, start=st, stop=sp,
                        skip_group_check=True,
                    )
                rb = rbp.tile([128, 512], F32, name="rb")
                nc.vector.reciprocal(rb[:], lb[:])
                nc.vector.tensor_mul(
                    outT16[:, tp * 512 : (tp + 1) * 512], pvb[:], rb[:]
                )

            return go

        def mk_fin(ib):
            def go():
                ps = mm_ps.tile([128, 512], F32, name="mm_ps")
                for t in range(4):
                    nc.tensor.matmul(
                        ps[:],
                        outT16[:, t * 512 + ib * 128 : t * 512 + (ib + 1) * 128],
                        wblk(3, t, 0, 512),
                        start=(t == 0),
                        stop=(t == 3),
                    )
                nc.vector.tensor_add(fin[:, ib * 512 : (ib + 1) * 512], ps[:], bo_sb[:])
                nc.sync.dma_start(
                    out[s, ib * 128 : (ib + 1) * 128],
                    fin[:, ib * 512 : (ib + 1) * 512],
                )

            return go

        return [
            mk_scores(0), mk_scores(1), mk_pv(0), mk_scores(2), mk_pv(1),
            mk_scores(3), mk_pv(2), mk_pv(3),
            mk_fin(0), mk_fin(1), mk_fin(2), mk_fin(3),
        ]

    # --- prologue: DMAs + slice-0 projections ---
    # weights go out on the scalar engine's DMA queue so they land in
    # parallel with the x loads on the sync queue
    emit_x_load(0)
    for widx in range(4):
        nc.scalar.dma_start(
            w16[:, widx * 2048 : (widx + 1) * 2048],
            w[widx].rearrange("(t p) e -> p t e", p=128),
        )
    nc.scalar.dma_start(bo_sb[:], bo_b)
    emit_x_load(1)
    for c in proj_chunks(0):
        c()
    if S > 2:
        emit_x_load(2)

    # --- pipelined slices: attn(s) interleaved with proj(s+1) ---
    # positions of proj chunks inside the 12 attention chunks: after S.tp1,
    # spread 2-at-a-time so PV.tpK never waits on ACT.
    interleave_after = {1: 2, 2: 2, 3: 2, 4: 2, 5: 2, 6: 1, 7: 1}
    for s in range(S):
        at = attn_chunks(s)
        nx = proj_chunks(s + 1) if s + 1 < S else []
        if s + 3 < S:
            emit_x_load(s + 3)
        ni = 0
        for i, c in enumerate(at):
            c()
            take = interleave_after.get(i, 0)
            for _ in range(take):
                if ni < len(nx):
                    nx[ni]()
                    ni += 1
        while ni < len(nx):
            nx[ni]()
            ni += 1


def build_nc():
    nc = bacc.Bacc("TRN2", target_bir_lowering=False, debug=False)
    x = nc.dram_tensor("x", [S, 2, DIM, N], BF16, kind="ExternalInput").ap()
    w = nc.dram_tensor("w", [4, DIM, DIM], BF16, kind="ExternalInput").ap()
    bo_b = nc.dram_tensor("bo_b", [128, DIM], F32, kind="ExternalInput").ap()
    out = nc.dram_tensor("out", [S, N, DIM], F32, kind="ExternalOutput").ap()
    with tile.TileContext(nc) as tc:
        with ExitStack() as ctx:
            _build_body(ctx, tc, x, w, bo_b, out)
    nc.compile()
    return nc


_NC = None
BF = ml_dtypes.bfloat16


def make_in_maps(q_in, kv_in, Wq, Wk, Wv, Wo, bo):
    # host-side layout prep: per-slice transpose to [dim, n] + bf16 cast + pack
    q = np.asarray(q_in, dtype=np.float32).reshape(32, N, DIM)
    kv = np.asarray(kv_in, dtype=np.float32).reshape(32, N, DIM)
    qT = np.ascontiguousarray(q.transpose(0, 2, 1)).astype(BF)
    kvT = np.ascontiguousarray(kv.transpose(0, 2, 1)).astype(BF)
    x_all = np.stack([qT, kvT], axis=1)  # [32, 2, DIM, N]
    w_all = np.stack(
        [np.asarray(a, dtype=np.float32) for a in (Wq, Wk, Wv, Wo)]
    ).astype(BF)
    bo_b = np.repeat(
        np.asarray(bo, dtype=np.float32)[None, :], 128, axis=0
    )  # [128, DIM]
    return [
        {"x": x_all[S * c : S * (c + 1)], "w": w_all, "bo_b": bo_b}
        for c in range(N_CORES)
    ]


def kernel(q_in, kv_in, Wq, Wk, Wv, Wo, bo):
    global _NC
    if _NC is None:
        _NC = build_nc()
    in_maps = make_in_maps(q_in, kv_in, Wq, Wk, Wv, Wo, bo)
    res = run_bass_kernel_spmd(_NC, in_maps, list(range(N_CORES))).results
    out = np.concatenate([res[c]["out"] for c in range(N_CORES)], axis=0)
    return out.reshape(4, 8, N, DIM)


# revision 3
# speedup vs baseline: 1.8840x; 1.0162x over previous
"""Trainium2 Bass kernel for CrossAttention (b=4, p=8, n=512, dim=512, 8 heads x 64).

Sharding: 32 independent (b, p) slices, 4 per core across 8 NeuronCores (pure
data parallel, no collectives). Weights replicated.

Host-side prep: activations transposed per-slice to [dim, n], cast bf16, and
packed with kv into one tensor; the 4 weight matrices packed into one tensor;
bo pre-broadcast to [128, 512] f32.

Device dataflow per slice (PE tile-array concurrency is the point):
  - projections qT/kT ([e, i]) and v ([j, e]) in 128x128 mode; q/k PSUM
    drained by DVE copies, v by ACT copies (engine load balance).
  - scores per head-pair (2*tp, 2*tp+1): K=64 row-tiled matmuls issued
    alternately at tile_position (0,0)/(64,0) so both heads stream
    concurrently in the 64x128 array config; exp on ACT (scale=1/8, no max
    subtraction: scores ~N(0,1)).
  - PV+l in 128x64 col-tiled mode: per jb, issue pv_even (cols 0-63),
    pv_odd (cols 64-127), l_even, l_odd alternating col groups; pv pair packs
    one PSUM bank (disjoint partitions), l pair another. l rows land aligned
    with their head's pv rows, so normalize is ONE DVE reciprocal + ONE
    tensor_mul over all 128 partitions per pair.
  - final projection 128x128 with bias added during the PSUM drain
    (DVE tensor_add with a pre-broadcast [128,512] bias tile) -> f32 -> DRAM.
  - emission is software-pipelined: slice s+1's projection chunks are
    interleaved into slice s's attention so the PE never waits on ACT.
"""

from contextlib import ExitStack

import ml_dtypes
import numpy as np

import concourse.bass as bass
import concourse.tile as tile
from concourse import bacc, mybir
from concourse.bass_utils import run_bass_kernel_spmd

F32 = mybir.dt.float32
BF16 = mybir.dt.bfloat16

HEADS = 8
DH = 64
N = 512
DIM = 512
SCALE = DH**-0.5
S = 4  # (b, p) slices per core
N_CORES = 8


def _build_body(ctx: ExitStack, tc: tile.TileContext, x, w, bo_b, out):
    nc = tc.nc

    const = ctx.enter_context(tc.tile_pool(name="const", bufs=1))
    xT = ctx.enter_context(tc.tile_pool(name="xT", bufs=2))
    proj = ctx.enter_context(tc.tile_pool(name="proj", bufs=2))
    ptp = ctx.enter_context(tc.tile_pool(name="ptp", bufs=4))
    outTp = ctx.enter_context(tc.tile_pool(name="outTp", bufs=2))
    rbp = ctx.enter_context(tc.tile_pool(name="rbp", bufs=2))
    finp = ctx.enter_context(tc.tile_pool(name="finp", bufs=2))
    mm_ps = ctx.enter_context(tc.tile_pool(name="mm_ps", bufs=2, space="PSUM"))
    st_ps = ctx.enter_context(tc.tile_pool(name="st_ps", bufs=2, space="PSUM"))
    pv_ps = ctx.enter_context(tc.tile_pool(name="pv_ps", bufs=2, space="PSUM"))
    l_ps = ctx.enter_context(tc.tile_pool(name="l_ps", bufs=2, space="PSUM"))

    # --- constants ---
    w16 = const.tile([128, 4 * 2048], BF16, name="w16")
    bo_sb = const.tile([128, 512], F32, name="bo_sb")
    ones64 = const.tile([128, 64], BF16, name="ones64")
    nc.gpsimd.memset(ones64[:], 1.0)

    def wblk(widx, d, lo, sz):
        # rows d-chunk (128 partitions), e-cols [lo, lo+sz) of weight widx
        base = widx * 2048 + d * 512 + lo
        return w16[:, base : base + sz]

    # per-slice persistent tiles
    xt_t = [None] * S

    def emit_x_load(s):
        t = xT.tile([128, 2 * 2048], BF16, name="xt")
        nc.sync.dma_start(t[:, 0:2048], x[s, 0].rearrange("(t p) n -> p t n", p=128))
        nc.sync.dma_start(
            t[:, 2048:4096], x[s, 1].rearrange("(t p) n -> p t n", p=128)
        )
        xt_t[s] = t

    qkv_t = [None] * S

    def proj_chunks(s):
        """12 emission chunks: q.t0-3, k.t0-3 ([e,i] layout), v.jb0-3 ([j,e])."""
        qT16 = proj.tile([128, 4 * 512], BF16, name="qT16")
        kT16 = proj.tile([128, 4 * 512], BF16, name="kT16")
        v16 = proj.tile([128, 4 * 512], BF16, name="v16")
        qkv_t[s] = (qT16, kT16, v16)
        xq = lambda d: xt_t[s][:, d * 512 : (d + 1) * 512]
        xkv = lambda d: xt_t[s][:, 2048 + d * 512 : 2048 + (d + 1) * 512]
        chunks = []

        def mk_qk(widx, t, dst):
            def go():
                ps = mm_ps.tile([128, 512], F32, name="mm_ps")
                for d in range(4):
                    nc.tensor.matmul(
                        ps[:],
                        wblk(widx, d, t * 128, 128),
                        xq(d) if widx == 0 else xkv(d),
                        start=(d == 0),
                        stop=(d == 3),
                    )
                nc.vector.tensor_copy(dst[:, t * 512 : (t + 1) * 512], ps[:])

            return go

        def mk_v(jb):
            def go():
                ps = mm_ps.tile([128, 512], F32, name="mm_ps")
                for d in range(4):
                    nc.tensor.matmul(
                        ps[:],
                        xkv(d)[:, jb * 128 : (jb + 1) * 128],
                        wblk(2, d, 0, 512),
                        start=(d == 0),
                        stop=(d == 3),
                    )
                nc.scalar.copy(v16[:, jb * 512 : (jb + 1) * 512], ps[:])

            return go

        for t in range(4):
            chunks.append(mk_qk(0, t, qT16))
        for t in range(4):
            chunks.append(mk_qk(1, t, kT16))
        for jb in range(4):
            chunks.append(mk_v(jb))
        return chunks

    def attn_chunks(s):
        """S.tp0, S.tp1, PV.tp0, S.tp2, PV.tp1, S.tp3, PV.tp2, PV.tp3,
        F.ib0-3 + store -> 12 chunks, emitted so each PV trails its exps."""
        qT16, kT16, v16 = qkv_t[s]
        outT16 = outTp.tile([128, 4 * 512], BF16, name="outT16")
        fin = finp.tile([128, 4 * 512], F32, name="fin")
        pts = [None] * 4

        def mk_scores(tp):
            def go():
                pt_e = ptp.tile([128, 4 * 512], BF16, name="pt_e")
                pt_o = ptp.tile([128, 4 * 512], BF16, name="pt_o")
                pts[tp] = (pt_e, pt_o)
                cols = slice(tp * 512, (tp + 1) * 512)
                for jb in range(4):
                    for half, pt in ((0, pt_e), (64, pt_o)):
                        kT_h = kT16[half : half + 64, cols]
                        qT_h = qT16[half : half + 64, cols]
                        stt = st_ps.tile([128, 512], F32, name="st_ps")
                        nc.tensor.matmul(
                            stt[:],
                            kT_h[:, jb * 128 : (jb + 1) * 128],
                            qT_h,
                            start=True,
                            stop=True,
                        )
                        nc.scalar.activation(
                            pt[:, jb * 512 : (jb + 1) * 512],
                            stt[:],
                            mybir.ActivationFunctionType.Exp,
                            scale=SCALE,
                        )

            return go

        def mk_pv(tp):
            h0, h1 = 2 * tp, 2 * tp + 1

            def go():
                pt_e, pt_o = pts[tp]
                pvb = pv_ps.tile([128, 512], F32, name="pv_ps")
                lb = l_ps.tile([128, 512], F32, name="l_ps")
                for jb in range(4):
                    st, sp = (jb == 0), (jb == 3)
                    pe_s = pt_e[:, jb * 512 : (jb + 1) * 512]
                    po_s = pt_o[:, jb * 512 : (jb + 1) * 512]
                    nc.tensor.matmul(
                        pvb[0:64, :],
                        v16[:, jb * 512 + h0 * 64 : jb * 512 + (h0 + 1) * 64],
                        pe_s, start=st, stop=sp, skip_group_check=True,
                    )
                    nc.tensor.matmul(
                        pvb[64:128, :],
                        v16[:, jb * 512 + h1 * 64 : jb * 512 + (h1 + 1) * 64],
                        po_s, start=st, stop=sp, skip_group_check=True,
                    )
                    nc.tensor.matmul(
                        lb[0:64, :], ones64[:], pe_s, start=st, stop=sp,
                        skip_group_check=True,
                    )
                    nc.tensor.matmul(
                        lb[64:128, :], ones64[:], po_s, start=st, stop=sp,
                        skip_group_check=True,
                    )
                rb = rbp.tile([128, 512], F32, name="rb")
                nc.vector.reciprocal(rb[:], lb[:])
                nc.vector.tensor_mul(
                    outT16[:, tp * 512 : (tp + 1) * 512], pvb[:], rb[:]
                )

            return go

        def mk_fin(ib):
            def go():
                ps = mm_ps.tile([128, 512], F32, name="mm_ps")
                for t in range(4):
                    nc.tensor.matmul(
                        ps[:],
                        outT16[:, t * 512 + ib * 128 : t * 512 + (ib + 1) * 128],
                        wblk(3, t, 0, 512),
                        start=(t == 0),
                        stop=(t == 3),
                    )
                nc.vector.tensor_add(fin[:, ib * 512 : (ib + 1) * 512], ps[:], bo_sb[:])
                nc.sync.dma_start(
                    out[s, ib * 128 : (ib + 1) * 128],
                    fin[:, ib * 512 : (ib + 1) * 512],
                )

            return go

        return [
            mk_scores(0), mk_scores(1), mk_pv(0), mk_scores(2), mk_pv(1),
            mk_scores(3), mk_pv(2), mk_pv(3),
            mk_fin(0), mk_fin(1), mk_fin(2), mk_fin(3),
        ]

    # --- prologue: DMAs + slice-0 projections ---
    # weights go out on the scalar engine's DMA queue so they land in
    # parallel with the x loads on the sync queue
    emit_x_load(0)
    for widx in range(4):
        nc.scalar.dma_start(
            w16[:, widx * 2048 : (widx + 1) * 2048],
            w[widx].rearrange("(t p) e -> p t e", p=128),
        )
    nc.scalar.dma_start(bo_sb[:], bo_b)
    emit_x_load(1)
    for c in proj_chunks(0):
        c()
    if S > 2:
        emit_x_load(2)

    # --- pipelined slices: attn(s) interleaved with proj(s+1) ---
    # positions of proj chunks inside the 12 attention chunks: after S.tp1,
    # spread 2-at-a-time so PV.tpK never waits on ACT.
    interleave_after = {1: 2, 2: 2, 3: 2, 4: 2, 5: 2, 6: 1, 7: 1}
    for s in range(S):
        at = attn_chunks(s)
        nx = proj_chunks(s + 1) if s + 1 < S else []
        if s + 3 < S:
            emit_x_load(s + 3)
        ni = 0
        for i, c in enumerate(at):
            c()
            take = interleave_after.get(i, 0)
            for _ in range(take):
                if ni < len(nx):
                    nx[ni]()
                    ni += 1
        while ni < len(nx):
            nx[ni]()
            ni += 1


def build_nc():
    nc = bacc.Bacc("TRN2", target_bir_lowering=False, debug=False)
    x = nc.dram_tensor("x", [S, 2, DIM, N], BF16, kind="ExternalInput").ap()
    w = nc.dram_tensor("w", [4, DIM, DIM], BF16, kind="ExternalInput").ap()
    bo_b = nc.dram_tensor("bo_b", [128, DIM], F32, kind="ExternalInput").ap()
    out = nc.dram_tensor("out", [S, N, DIM], F32, kind="ExternalOutput").ap()
    with tile.TileContext(nc) as tc:
        with ExitStack() as ctx:
            _build_body(ctx, tc, x, w, bo_b, out)
    nc.compile()
    return nc


_NC = None
BF = ml_dtypes.bfloat16


def make_in_maps(q_in, kv_in, Wq, Wk, Wv, Wo, bo):
    # host-side layout prep: per-slice transpose to [dim, n] + bf16 cast + pack
    q = np.asarray(q_in, dtype=np.float32).reshape(32, N, DIM)
    kv = np.asarray(kv_in, dtype=np.float32).reshape(32, N, DIM)
    qT = np.ascontiguousarray(q.transpose(0, 2, 1)).astype(BF)
    kvT = np.ascontiguousarray(kv.transpose(0, 2, 1)).astype(BF)
    x_all = np.stack([qT, kvT], axis=1)  # [32, 2, DIM, N]
    w_all = np.stack(
        [np.asarray(a, dtype=np.float32) for a in (Wq, Wk, Wv, Wo)]
    ).astype(BF)
    bo_b = np.repeat(
        np.asarray(bo, dtype=np.float32)[None, :], 128, axis=0
    )  # [128, DIM]
    return [
        {"x": x_all[S * c : S * (c + 1)], "w": w_all, "bo_b": bo_b}
        for c in range(N_CORES)
    ]


def kernel(q_in, kv_in, Wq, Wk, Wv, Wo, bo):
    global _NC
    if _NC is None:
        _NC = build_nc()
    in_maps = make_in_maps(q_in, kv_in, Wq, Wk, Wv, Wo, bo)
    res = run_bass_kernel_spmd(_NC, in_maps, list(range(N_CORES))).results
    out = np.concatenate([res[c]["out"] for c in range(N_CORES)], axis=0)
    return out.reshape(4, 8, N, DIM)
